# revision 7
# baseline (speedup 1.0000x reference)
"""Trainium2 Bass kernel for nn_CubicSpline (natural cubic spline radial eval).

Formulation: out[t, ch] = sum_s Theta[s, ch] * V_s(u_t), u = r/h, where the
V rows are truncated |.|-cubes  relu(w - |u - c|)^3  at two radii (w=2, w=1),
whose span contains the cubic B-spline bumps and hence every natural cubic
spline on the integer knot grid exactly (fp32 residual ~5e-7, |theta| <= ~6).

Device pipeline per 512-trial block (channel-major PSUM output), all fp32
(f32r was measured at ~1e-3 rel err vs 2.6e-5 for fp32 - rejected):
  PE   mm1: u_bcast[128,512] = (ones/h).T @ r_row      (K=1 fp32 matmul)
  DVE  passA: VA = relu(2 - |u - cA|)^3                (one custom op, 7 stages)
  DVE  passV: VV = relu(1 - |u - cV|)^3                (same op, other params)
  PE   out_psum = ThA.T @ VA + ThV.T @ VV              (2x K=128 fp32 matmuls)
  ACT  evict: out_sbuf = Identity(out_psum + bias)     (per-channel bias row)
  DMA  out_sbuf -> HBM shard [128, Nc] (channel-major; host transposes)

Data-parallel over 8 NeuronCores: r sharded along N, theta tables replicated.
"""

import os
import numpy as np

N_TOTAL = 2_000_000
N_CORES = 8
N_KNOTS = 128
RMAX = 6.0
H = RMAX / (N_KNOTS - 1)
BLK = 512
NC_RAW = N_TOTAL // N_CORES                 # 250_000
BLOCKS = (NC_RAW + BLK - 1) // BLK          # 489
NC_PAD = BLOCKS * BLK                       # 250_368
CHUNK_BLKS = 16
USE_GPSIMD_BCAST = False

_PROGRAM_CACHE = {}


def _register_op():
    from concourse import dve_ops
    from concourse.dve_spec import Spec, Src0, C0, C1, Zero, relu, sq, maxx, lower
    from concourse.dve_uop import DveOpSpec

    for o in dve_ops.OPS:
        if o.name == "BUMP3_ANT":
            return o
    t = Src0 - C0
    y = maxx(t, Zero - t)
    m = relu(C1 - y)
    spec = Spec(
        body=sq(m) * m,
        reference=lambda in0, s0, s1, imm2=0.0: np.maximum(
            s1 - np.abs(in0 - s0), 0.0
        ).astype(np.float32) ** 3,
    )
    op = dve_ops.DveOp("BUMP3_ANT", spec, subdim=False, uops_sha={})
    _append_op(dve_ops, op, spec, DveOpSpec, lower)
    return op


def _append_op(dve_ops, op, spec, DveOpSpec, lower):
    dve_ops.OPS.append(op)
    dve_ops._SUB_OPCODE_FOR_NAME[op.name] = (
        dve_ops._CUSTOM_DVE_ROW_BASE + len(dve_ops.OPS) - 1
    )
    dve_ops.CUSTOM_DVE_SPECS[op.name] = op.spec
    for ver in ("v3", "v4"):
        try:
            uops = lower(spec, ver=ver)
            op.uops_sha[ver] = DveOpSpec(
                name=op.name,
                opcode=dve_ops.get_dve_sub_opcode(op.name),
                uops=uops,
                rd1_en=False,
            ).sha(ver)
        except Exception:
            pass


def _register_op_scaled():
    """BUMP3S: m = relu(s1 - |in0*imm2 - s0|); out = m^3  (scale folded in)."""
    from concourse import dve_ops
    from concourse.dve_spec import Spec, Src0, C0, C1, C2, Zero, relu, sq, maxx, lower
    from concourse.dve_uop import DveOpSpec

    for o in dve_ops.OPS:
        if o.name == "BUMP3S_ANT":
            return o
    t = Src0 * C2 - C0
    y = maxx(t, Zero - t)
    m = relu(C1 - y)
    spec = Spec(
        body=sq(m) * m,
        reference=lambda in0, s0, s1, imm2: np.maximum(
            s1 - np.abs(in0 * imm2 - s0), 0.0
        ).astype(np.float32) ** 3,
    )
    op = dve_ops.DveOp("BUMP3S_ANT", spec, subdim=False, uops_sha={})
    _append_op(dve_ops, op, spec, DveOpSpec, lower)
    return op


# basis row centers (in u = r/h units)
CT_A = np.arange(-1, 127, dtype=np.float64)   # radius-2 rows, ct = -1..126
CT_V = np.arange(0, 128, dtype=np.float64)    # radius-1 rows, ct = 0..127


def _solve_theta(coefficients):
    """Fit bias + 256 cube rows to the spline defined by `coefficients`."""
    coef = np.asarray(coefficients, np.float64)           # [127, 4, 128]
    segs = np.arange(127)
    ts = (np.arange(8) + 0.5) / 8
    u = (segs[:, None] + ts[None, :]).ravel()             # [1016]
    idx = np.clip(np.floor(u).astype(int), 0, 126)
    dx = (u - idx) * H
    a, b, c, d = (coef[idx, k] for k in range(4))
    P = a + dx[:, None] * (b + dx[:, None] * (c + dx[:, None] * d))  # [1016,128]

    B = np.empty((u.size, 257))
    B[:, 0] = 1.0
    for i, ct in enumerate(CT_A):
        m = np.maximum(2.0 - np.abs(u - ct), 0.0)
        B[:, 1 + i] = m * m * m
    for i, ct in enumerate(CT_V):
        m = np.maximum(1.0 - np.abs(u - ct), 0.0)
        B[:, 129 + i] = m * m * m
    theta, _, _, _ = np.linalg.lstsq(B, P, rcond=None)
    bias = theta[0].astype(np.float32).reshape(128, 1)
    thA = theta[1:129].astype(np.float32)                 # [128 rows, 128 ch]
    thV = theta[129:257].astype(np.float32)
    return thA, thV, bias


def _build_program(n_blocks):
    if n_blocks in _PROGRAM_CACHE:
        return _PROGRAM_CACHE[n_blocks]
    import concourse.bacc as bacc
    import concourse.mybir as mybir
    from concourse.tile import TileContext

    op = _register_op()
    ops = _register_op_scaled()
    f32 = mybir.dt.float32
    f32r = mybir.dt.float32r
    nc = bacc.Bacc(
        "TRN2", target_bir_lowering=False, debug=False, num_devices=N_CORES
    )
    n_pad = n_blocks * BLK
    r_ap = nc.dram_tensor("r", [1, n_pad], f32, kind="ExternalInput").ap()
    thA_ap = nc.dram_tensor("thA", [128, 128], f32, kind="ExternalInput").ap()
    thV_ap = nc.dram_tensor("thV", [128, 128], f32, kind="ExternalInput").ap()
    bias_ap = nc.dram_tensor("bias", [128, 1], f32, kind="ExternalInput").ap()
    ctA_ap = nc.dram_tensor("ctA", [128, 1], f32, kind="ExternalInput").ap()
    ctV_ap = nc.dram_tensor("ctV", [128, 1], f32, kind="ExternalInput").ap()
    ones_ap = nc.dram_tensor("onesh", [1, 128], f32, kind="ExternalInput").ap()
    out_ap = nc.dram_tensor("out", [128, n_pad], f32, kind="ExternalOutput").ap()

    with TileContext(nc) as tc:
        with tc.tile_pool(name="const", bufs=1) as cpool, tc.tile_pool(
            name="work", bufs=3
        ) as pool, tc.tile_pool(name="rch", bufs=2) as rpool, tc.tile_pool(
            name="pu", bufs=2, space="PSUM"
        ) as ppool, tc.tile_pool(name="po", bufs=2, space="PSUM") as opool:
            thA_t = cpool.tile([128, 128], f32)
            nc.sync.dma_start(thA_t[:], thA_ap)
            thV_t = cpool.tile([128, 128], f32)
            nc.sync.dma_start(thV_t[:], thV_ap)
            bias_t = cpool.tile([128, 1], f32)
            nc.sync.dma_start(bias_t[:], bias_ap)
            ctA_t = cpool.tile([128, 1], f32)
            nc.sync.dma_start(ctA_t[:], ctA_ap)
            ctV_t = cpool.tile([128, 1], f32)
            nc.sync.dma_start(ctV_t[:], ctV_ap)
            ones_t = cpool.tile([1, 128], f32)
            nc.sync.dma_start(ones_t[:], ones_ap)

            for c0 in range(0, n_blocks, CHUNK_BLKS):
                bc = min(CHUNK_BLKS, n_blocks - c0)
                rch = rpool.tile([1, CHUNK_BLKS * BLK], f32, tag="rch")
                nc.sync.dma_start(
                    rch[:, : bc * BLK], r_ap[:, c0 * BLK : (c0 + bc) * BLK]
                )
                for b in range(bc):
                    rsl = rch[:, b * BLK : (b + 1) * BLK]
                    if USE_GPSIMD_BCAST:
                        pu = pool.tile([128, BLK], f32, tag="pu")
                        nc.gpsimd.partition_broadcast(pu[:], rsl, channels=128)
                        inv_h = float(np.float32(1.0) / np.float32(H))
                        va = pool.tile([128, BLK], f32, tag="va")
                        nc.vector._custom_dve(
                            ops, out=va[:], in0=pu[:], s0=ctA_t[:], s1=2.0, imm2=inv_h
                        )
                        vv = pool.tile([128, BLK], f32, tag="vv")
                        nc.vector._custom_dve(
                            ops, out=vv[:], in0=pu[:], s0=ctV_t[:], s1=1.0, imm2=inv_h
                        )
                    else:
                        pu = ppool.tile([128, BLK], f32, tag="pu")
                        nc.tensor.matmul(
                            pu[:],
                            ones_t[:],
                            rsl,
                            start=True,
                            stop=True,
                        )
                        va = pool.tile([128, BLK], f32, tag="va")
                        nc.vector._custom_dve(
                            op, out=va[:], in0=pu[:], s0=ctA_t[:], s1=2.0
                        )
                        vv = pool.tile([128, BLK], f32, tag="vv")
                        nc.vector._custom_dve(
                            op, out=vv[:], in0=pu[:], s0=ctV_t[:], s1=1.0
                        )
                    po = opool.tile([128, BLK], f32, tag="po")
                    nc.tensor.matmul(
                        po[:],
                        thA_t[:],
                        va[:],
                        start=True,
                        stop=False,
                    )
                    nc.tensor.matmul(
                        po[:],
                        thV_t[:],
                        vv[:],
                        start=False,
                        stop=True,
                    )
                    ob = pool.tile([128, BLK], f32, tag="ob")
                    nc.scalar.activation(
                        ob[:],
                        po[:],
                        mybir.ActivationFunctionType.Identity,
                        bias=bias_t[:],
                        scale=1.0,
                    )
                    blk = c0 + b
                    nc.sync.dma_start(out_ap[:, blk * BLK : (blk + 1) * BLK], ob[:])
    nc.compile()
    _PROGRAM_CACHE[n_blocks] = nc
    return nc


def kernel(r_trial, r_knots, coefficients, h, rmax):
    r = np.ascontiguousarray(np.asarray(r_trial, np.float32))
    n = r.shape[0]
    thA, thV, bias = _solve_theta(coefficients)
    inv_h = np.float32(1.0 / H)

    n_blocks = BLOCKS
    n_pad = NC_PAD
    r_pad = np.zeros(N_CORES * n_pad, np.float32)
    r_pad[:n] = r
    shards = r_pad.reshape(N_CORES, 1, n_pad)

    ctA32 = (CT_A.astype(np.float32)).reshape(128, 1)
    ctV32 = (CT_V.astype(np.float32)).reshape(128, 1)
    ones = np.full((1, 128), inv_h, np.float32)

    nc = _build_program(n_blocks)
    in_maps = [
        {
            "r": shards[i],
            "thA": thA,
            "thV": thV,
            "bias": bias,
            "ctA": ctA32,
            "ctV": ctV32,
            "onesh": ones,
        }
        for i in range(N_CORES)
    ]
    from concourse.bass_utils import run_bass_kernel_spmd

    res = run_bass_kernel_spmd(nc, in_maps, core_ids=list(range(N_CORES)))
    full = np.empty((N_CORES * n_pad, 128), np.float32)
    for i in range(N_CORES):
        full[i * n_pad : (i + 1) * n_pad] = res.results[i]["out"].T
    return full[:n]
